# revision 21
# baseline (speedup 1.0000x reference)
"""Trainium2 Bass kernel for a custom transformer block (v2).

Sharding: 8 cores = 4 batches x 2 query-interleave classes. Core (b, c)
owns global 128-query blocks gb = 2L + c (L = 0..7), so the causal
triangle splits evenly: per-L key-chunk counts CNT = [2,4,6,8,8,8,8,8]
are identical across cores and 18.75% of score/exp/AV work is skipped.

Key techniques vs v1:
- All matmuls in bf16 (weights pre-cast host-side; LN gains/biases and
  the 1/sqrt(D) scale folded into wq/wkv/w1 on the host).
- No PE transposes: XBAR DMA-transpose moves z -> zT, probs -> probsT,
  attention-out -> token-major, z2 -> z2T.
- Scores computed query-major [q, keys] per (head, L); key-pad mask
  folded into the score matmul via a 65th contraction row (q_aux = 1,
  k_aux = -80 * pad). Causal masking only on the two diagonal 128-key
  chunks per L < 4, as a {0,1} bf16 multiply after exp.
- No softmax division on the full tensor: per-(q, head) denominators
  are reduced from the bf16 probs (DVE) and applied as a single [P,1]
  scale on the 64-wide attention output (Pool).
- Engine split: PE matmuls; ACT exp/silu/QK evacs; DVE stats, dens,
  oa+h2 psum evacs; Pool SBUF-side applies/masks/normalize (GPSIMD
  cannot touch PSUM); SP issues all DMAs.
"""
import sys
import os

if "/opt/trn_rl_repo" not in sys.path:
    sys.path.insert(0, "/opt/trn_rl_repo")

import numpy as np
import ml_dtypes

B, S, D = 4, 2048, 1024
N_HEAD = 16
D_HEAD = 64
WINDOW = 1024
D_FF = 4096
EPS = 1e-5
P = 128
CNT = [2, 4, 6, 8, 8, 8, 8, 8]
MASKVAL = -80.0

_CACHE = {}


def _build_program():
    import concourse.bacc as bacc
    import concourse.mybir as mybir
    from concourse.tile import TileContext

    F32 = mybir.dt.float32
    BF16 = mybir.dt.bfloat16
    AF = mybir.ActivationFunctionType
    ALU = mybir.AluOpType
    AX = mybir.AxisListType

    nc = bacc.Bacc("TRN2", target_bir_lowering=False, debug=False,
                   num_devices=8)

    xin_d = nc.dram_tensor("xin", [2 * WINDOW, D], BF16, kind="ExternalInput")
    xinT_d = nc.dram_tensor("xinT", [D, WINDOW], BF16, kind="ExternalInput")
    wq_d = nc.dram_tensor("wq", [D, D], BF16, kind="ExternalInput")
    wkv_d = nc.dram_tensor("wkv", [D, 2 * D], BF16, kind="ExternalInput")
    w1_d = nc.dram_tensor("w1", [D, D_FF], BF16, kind="ExternalInput")
    w2_d = nc.dram_tensor("w2", [D_FF, D], BF16, kind="ExternalInput")
    bqs_d = nc.dram_tensor("bqs", [P, 8], F32, kind="ExternalInput")
    bkvk_d = nc.dram_tensor("bkvk", [P, 8], F32, kind="ExternalInput")
    bvb_d = nc.dram_tensor("bvb", [P, D], F32, kind="ExternalInput")
    b1s_d = nc.dram_tensor("b1s", [P, 32], F32, kind="ExternalInput")
    b2s_d = nc.dram_tensor("b2s", [P, 8], F32, kind="ExternalInput")
    qrow_d = nc.dram_tensor("qrow", [1, N_HEAD * WINDOW], BF16,
                            kind="ExternalInput")
    krow_d = nc.dram_tensor("krow", [1, N_HEAD * WINDOW], BF16,
                            kind="ExternalInput")
    maskq_d = nc.dram_tensor("maskq", [P, 256], BF16, kind="ExternalInput")
    y_d = nc.dram_tensor("y", [D, WINDOW], F32, kind="ExternalOutput")

    with TileContext(nc) as tc:
        cpool = tc.alloc_tile_pool(name="const", bufs=1, side="left")
        smallc = cpool.tile([P, 56], F32)
        bqs = smallc[:, 0:8]
        bkvk = smallc[:, 8:16]
        b1s = smallc[:, 16:48]
        b2s = smallc[:, 48:56]
        bvb = cpool.tile([P, D], F32)
        maskq = cpool.tile([P, 256], BF16)
        nc.sync.dma_start(bqs, bqs_d[:])
        nc.sync.dma_start(bkvk, bkvk_d[:])
        nc.sync.dma_start(b1s, b1s_d[:])
        nc.sync.dma_start(b2s, b2s_d[:])
        nc.sync.dma_start(bvb[:], bvb_d[:])
        nc.sync.dma_start(maskq[:], maskq_d[:])

        # ---------------- LN1 + DMA-transpose to dim-major -----------------
        zTp = tc.alloc_tile_pool(name="zT", bufs=1, side="left")
        zqT = zTp.tile([P, 8, WINDOW], BF16)
        zwT = zTp.tile([P, 8, WINDOW], BF16)
        xz = tc.alloc_tile_pool(name="xz", bufs=3, side="left")

        def ln1_tile(t):
            xt = xz.tile([P, D], BF16, tag="x")
            nc.sync.dma_start(xt[:], xin_d[t * P:(t + 1) * P, :])
            st = xz.tile([P, 8], F32, tag="stats")
            musum, mu, vsum = st[:, 0:1], st[:, 1:2], st[:, 2:3]
            veps, sdv, rstd = st[:, 4:5], st[:, 5:6], st[:, 6:7]
            nc.vector.reduce_sum(musum, xt[:], axis=AX.X)
            nc.vector.tensor_scalar_mul(mu, musum, 1.0 / D)
            ztmp = xz.tile([P, D], BF16, tag="ztmp")
            nc.vector.scalar_tensor_tensor(
                ztmp[:], xt[:], mu, xt[:],
                op0=ALU.subtract, op1=ALU.mult, accum_out=vsum)
            nc.vector.tensor_scalar(veps, vsum, 1.0 / D, EPS,
                                    op0=ALU.mult, op1=ALU.add)
            nc.scalar.sqrt(sdv, veps)
            nc.vector.reciprocal(rstd, sdv)
            z = xz.tile([P, D], BF16, tag="z")
            nc.vector.tensor_scalar(z[:], xt[:], mu, rstd,
                                    op0=ALU.subtract, op1=ALU.mult)
            if t < 8:
                dst = zqT[:, :, t * P:(t + 1) * P]
            else:
                dst = zwT[:, :, (t - 8) * P:(t - 7) * P]
            nc.sync.dma_start(dst, z[:], transpose=True)

        qkvp = tc.alloc_tile_pool(name="qkv", bufs=1, side="right")
        qT = qkvp.tile([P, N_HEAD, WINDOW], BF16)   # rows 0-63 dims, 64 ones
        kT = qkvp.tile([P, N_HEAD, WINDOW], BF16)   # rows 0-63 dims, 64 -80*pad
        V = qkvp.tile([P, 8, D], BF16)              # token-major

        wst = tc.alloc_tile_pool(name="wst", bufs=2, side="left")
        psC = tc.alloc_tile_pool(name="psC", bufs=4, space="PSUM")

        for t in range(8):
            ln1_tile(t)

        # Q: weights stationary -> psum [dims 128, tokens 512]
        # qh-outer so qh0 matmuls start after only 4 LN tiles
        wqrs = []
        for wh in range(2):
            wqr = wst.tile([P, 8, 512], BF16, tag="wchunk")
            for kc in range(8):
                nc.sync.dma_start(
                    wqr[:, kc, :],
                    wq_d[kc * P:(kc + 1) * P, wh * 512:(wh + 1) * 512])
            wqrs.append(wqr)
        for qh in range(2):
            for co in range(8):
                pp = psC.tile([P, 512], F32, tag="proj")
                for kc in range(8):
                    nc.tensor.matmul(
                        pp[:], wqrs[co // 4][:, kc, (co % 4) * P:(co % 4 + 1) * P],
                        zqT[:, kc, qh * 512:(qh + 1) * 512],
                        start=(kc == 0), stop=(kc == 7))
                for hh in range(2):
                    nc.scalar.activation(
                        qT[0:64, 2 * co + hh, qh * 512:(qh + 1) * 512],
                        pp[hh * 64:hh * 64 + 64, :], AF.Identity,
                        bias=bqs[hh * 64:hh * 64 + 64, co:co + 1],
                        scale=1.0)
        nc.sync.dma_start(qT[64:65, :, :], qrow_d[:])

        for t in range(8, 16):
            ln1_tile(t)

        # V: activations stationary -> psum [keys 128, dims 512]
        for vh in range(2):
            wvr = wst.tile([P, 8, 512], BF16, tag="wchunk")
            for kc in range(8):
                nc.sync.dma_start(
                    wvr[:, kc, :],
                    wkv_d[kc * P:(kc + 1) * P,
                          D + vh * 512:D + (vh + 1) * 512])
            for tt in range(8):
                pp = psC.tile([P, 512], F32, tag="proj")
                for kc in range(8):
                    nc.tensor.matmul(
                        pp[:], zwT[:, kc, tt * P:(tt + 1) * P],
                        wvr[:, kc, :],
                        start=(kc == 0), stop=(kc == 7))
                nc.vector.tensor_tensor(
                    V[:, tt, vh * 512:(vh + 1) * 512], pp[:],
                    bvb[:, vh * 512:(vh + 1) * 512], op=ALU.add)

        # K: weights stationary
        for wh in range(2):
            wkr = wst.tile([P, 8, 512], BF16, tag="wchunk")
            for kc in range(8):
                nc.sync.dma_start(
                    wkr[:, kc, :],
                    wkv_d[kc * P:(kc + 1) * P, wh * 512:(wh + 1) * 512])
            for co in range(wh * 4, wh * 4 + 4):
                for qh in range(2):
                    pp = psC.tile([P, 512], F32, tag="proj")
                    for kc in range(8):
                        nc.tensor.matmul(
                            pp[:], wkr[:, kc, (co % 4) * P:(co % 4 + 1) * P],
                            zwT[:, kc, qh * 512:(qh + 1) * 512],
                            start=(kc == 0), stop=(kc == 7))
                    for hh in range(2):
                        nc.scalar.activation(
                            kT[0:64, 2 * co + hh, qh * 512:(qh + 1) * 512],
                            pp[hh * 64:hh * 64 + 64, :], AF.Identity,
                            bias=bkvk[hh * 64:hh * 64 + 64, co:co + 1],
                            scale=1.0)
        nc.sync.dma_start(kT[64:65, :, :], krow_d[:])

        psC.release()
        wst.release()
        xz.release()
        zTp.release()

        # ---------------- attention -----------------------------------------
        attnp = tc.alloc_tile_pool(name="attn", bufs=1, side="left")
        attn = attnp.tile([P, 8, D], BF16)
        den = attnp.tile([P, 8, N_HEAD], F32)
        rinva = attnp.tile([P, 8, N_HEAD], F32)

        # MLP sc0 weight prefetch (PE can start MLP right after LN2)
        wf1 = tc.alloc_tile_pool(name="wf1", bufs=1, side="left")
        w1r0 = wf1.tile([P, 8, 1024], BF16, tag="w1r")
        for kc in range(8):
            nc.sync.dma_start(w1r0[:, kc, :],
                              w1_d[kc * P:(kc + 1) * P, 0:1024])

        pqp = tc.alloc_tile_pool(name="pq", bufs=3, side="left")
        ptp = tc.alloc_tile_pool(name="ptsT", bufs=3, side="left")
        oap = tc.alloc_tile_pool(name="oa", bufs=2, side="left")
        psS = tc.alloc_tile_pool(name="psS", bufs=3, space="PSUM")
        psA = tc.alloc_tile_pool(name="psA", bufs=2, space="PSUM")

        steps = [(hp, L) for hp in range(8) for L in range(8)]
        pts_saved = {}
        oa_tiles = {}

        def emit_scores_pair(sA, sB):
            # two steps share one pq tile and ONE XBAR transpose (SP issue
            # of DMA_TRANSPOSE is ~2us, so halve the count)
            pq = pqp.tile([P, 4096], BF16, tag="pq")
            off = 0
            offs = {}
            for (hp, L) in (sA, sB):
                nk = CNT[L] * P
                offs[(hp, L)] = off
                for hh in range(2):
                    h = 2 * hp + hh
                    sps = psS.tile([P, 1024], F32, tag="s")
                    for half in range((nk + 511) // 512):
                        w = min(512, nk - half * 512)
                        nc.tensor.matmul(
                            sps[:, half * 512:half * 512 + w],
                            qT[0:65, h, L * P:(L + 1) * P],
                            kT[0:65, h, half * 512:half * 512 + w],
                            start=True, stop=True)
                    o = off + hh * nk
                    if L >= 4:
                        # no causal mask: denominator rides the ACT accum
                        nc.scalar.activation(pq[:, o:o + nk],
                                             sps[:, 0:nk], AF.Exp,
                                             accum_out=den[:, L, h:h + 1])
                    else:
                        nc.scalar.activation(pq[:, o:o + nk],
                                             sps[:, 0:nk], AF.Exp)
                        nc.vector.tensor_tensor(
                            pq[:, o + 2 * L * P:o + nk],
                            pq[:, o + 2 * L * P:o + nk],
                            maskq[:], op=ALU.mult)
                        nc.vector.reduce_sum(den[:, L, h:h + 1],
                                             pq[:, o:o + nk], axis=AX.X)
                off += 2 * nk
            pt = ptp.tile([P, 32, P], BF16, tag="pt")
            nc.sync.dma_start(pt[:, 0:off // P, :], pq[:, 0:off],
                              transpose=True)
            pts_saved[sA] = (pt, offs[sA] // P)
            pts_saved[sB] = (pt, offs[sB] // P)

        def emit_av(hp, L):
            cnt = CNT[L]
            if L == 0:
                oa = oap.tile([P, WINDOW], BF16, tag="oa")
                oa_tiles[hp] = oa
            oa = oa_tiles[hp]
            pt, mb = pts_saved.pop((hp, L))
            avp = psA.tile([P, P], F32, tag="av")
            for hh in range(2):
                h = 2 * hp + hh
                for kc in range(cnt):
                    nc.tensor.matmul(
                        avp[hh * 64:hh * 64 + 64, :],
                        V[:, kc, h * 64:(h + 1) * 64],
                        pt[:, mb + cnt * hh + kc, :],
                        start=(kc == 0), stop=(kc == cnt - 1))
            nc.vector.tensor_copy(oa[:, L * P:(L + 1) * P], avp[:])
            if L == 7:
                nc.sync.dma_start(attn[:, :, hp * P:(hp + 1) * P], oa[:],
                                  transpose=True)

        pairs = [(steps[i], steps[i + 1]) for i in range(0, len(steps), 2)]
        for j, pr in enumerate(pairs):
            emit_scores_pair(*pr)
            if j >= 1:
                emit_av(*pairs[j - 1][0])
                emit_av(*pairs[j - 1][1])
        emit_av(*pairs[-1][0])
        emit_av(*pairs[-1][1])

        nc.vector.reciprocal(rinva[:], den[:])

        psA.release()
        psS.release()
        oap.release()
        ptp.release()
        pqp.release()
        qkvp.release()

        # ---------------- normalize + LN2 + DMA-transpose -------------------
        z2Tp = tc.alloc_tile_pool(name="z2T", bufs=1, side="right")
        z2T = z2Tp.tile([P, 8, WINDOW], BF16)
        xz2 = tc.alloc_tile_pool(name="xz2", bufs=3, side="left")

        for t in range(8):
            at = attn[:, t, :]
            for h in range(N_HEAD):
                nc.scalar.activation(
                    at[:, h * 64:(h + 1) * 64], at[:, h * 64:(h + 1) * 64],
                    AF.Copy, scale=rinva[:, t, h:h + 1])
            st = xz2.tile([P, 8], F32, tag="stats2")
            musum, mu, vsum = st[:, 0:1], st[:, 1:2], st[:, 2:3]
            veps, sdv, rstd = st[:, 4:5], st[:, 5:6], st[:, 6:7]
            nc.vector.reduce_sum(musum, at, axis=AX.X)
            nc.vector.tensor_scalar_mul(mu, musum, 1.0 / D)
            ztmp = xz2.tile([P, D], BF16, tag="z2tmp")
            nc.vector.scalar_tensor_tensor(
                ztmp[:], at, mu, at,
                op0=ALU.subtract, op1=ALU.mult, accum_out=vsum)
            nc.vector.tensor_scalar(veps, vsum, 1.0 / D, EPS,
                                    op0=ALU.mult, op1=ALU.add)
            nc.scalar.sqrt(sdv, veps)
            nc.vector.reciprocal(rstd, sdv)
            z2 = xz2.tile([P, D], BF16, tag="z2")
            nc.vector.tensor_scalar(z2[:], at, mu, rstd,
                                    op0=ALU.subtract, op1=ALU.mult)
            nc.sync.dma_start(z2T[:, :, t * P:(t + 1) * P], z2[:],
                              transpose=True)

        # ---------------- MLP ------------------------------------------------
        h2p = tc.alloc_tile_pool(name="h2acc", bufs=1, side="left")
        h2acc = h2p.tile([P, 8, WINDOW], F32)
        xinTp = tc.alloc_tile_pool(name="xinT", bufs=1, side="left")
        xinT = xinTp.tile([P, 8, WINDOW], BF16)
        nc.sync.dma_start(xinT[:], xinT_d.rearrange("(c p) n -> p c n", p=P))
        for co in range(8):
            nc.vector.tensor_scalar(h2acc[:, co, :], xinT[:, co, :],
                                    b2s[:, co:co + 1], None, op0=ALU.add)

        wf2 = tc.alloc_tile_pool(name="wf2", bufs=2, side="right")
        h1p = tc.alloc_tile_pool(name="h1p", bufs=1, side="left")
        psF1 = tc.alloc_tile_pool(name="psF1", bufs=4, space="PSUM")
        psF2 = tc.alloc_tile_pool(name="psF2", bufs=4, space="PSUM")

        for sc in range(4):
            if sc == 0:
                w1r = w1r0
            else:
                w1r = wf1.tile([P, 8, 1024], BF16, tag="w1r")
                for kc in range(8):
                    nc.sync.dma_start(
                        w1r[:, kc, :],
                        w1_d[kc * P:(kc + 1) * P, sc * 1024:(sc + 1) * 1024])
            h1 = h1p.tile([P, 8, WINDOW], BF16, tag="h1")
            for qh in range(2):
                for ft in range(8):
                    hp1 = psF1.tile([P, 512], F32, tag="h1ps")
                    for kc in range(8):
                        nc.tensor.matmul(
                            hp1[:], w1r[:, kc, ft * P:(ft + 1) * P],
                            z2T[:, kc, qh * 512:(qh + 1) * 512],
                            start=(kc == 0), stop=(kc == 7))
                    nc.scalar.activation(
                        h1[:, ft, qh * 512:(qh + 1) * 512], hp1[:], AF.Silu,
                        bias=b1s[:, sc * 8 + ft:sc * 8 + ft + 1], scale=1.0)
            w2r = wf2.tile([P, 8, 1024], BF16, tag="w2r")
            for kc in range(8):
                nc.sync.dma_start(
                    w2r[:, kc, :],
                    w2_d[(sc * 8 + kc) * P:(sc * 8 + kc + 1) * P, :])
            for co in range(8):
                for qh in range(2):
                    hp2 = psF2.tile([P, 512], F32, tag="h2ps")
                    for kc in range(8):
                        nc.tensor.matmul(
                            hp2[:], w2r[:, kc, co * P:(co + 1) * P],
                            h1[:, kc, qh * 512:(qh + 1) * 512],
                            start=(kc == 0), stop=(kc == 7))
                    dstp = h2acc[:, co, qh * 512:(qh + 1) * 512]
                    nc.vector.tensor_tensor(dstp, hp2[:], dstp, op=ALU.add)
                if sc == 3:
                    nc.sync.dma_start(y_d[co * P:(co + 1) * P, :],
                                      h2acc[:, co, :])

        psF2.release()
        psF1.release()
        h1p.release()
        wf2.release()
        xinTp.release()
        h2p.release()
        xz2.release()
        z2Tp.release()
        wf1.release()
        attnp.release()
        cpool.release()

    nc.compile()
    return nc


def _prep_inputs(inputs):
    bfd = ml_dtypes.bfloat16
    x = np.asarray(inputs["x"], dtype=np.float32)
    kpm = np.asarray(inputs["key_pad_mask"]).astype(bool)
    wq = np.asarray(inputs["wq"], dtype=np.float32)
    wkv = np.asarray(inputs["wkv"], dtype=np.float32)
    w1 = np.asarray(inputs["w1"], dtype=np.float32)
    w2 = np.asarray(inputs["w2"], dtype=np.float32)
    bq = np.asarray(inputs["bq"], dtype=np.float32)
    bkv = np.asarray(inputs["bkv"], dtype=np.float32)
    b1 = np.asarray(inputs["b1"], dtype=np.float32)
    b2 = np.asarray(inputs["b2"], dtype=np.float32)
    g1 = np.asarray(inputs["ln1_g"], dtype=np.float32)
    lb1 = np.asarray(inputs["ln1_b"], dtype=np.float32)
    g2 = np.asarray(inputs["ln2_g"], dtype=np.float32)
    lb2 = np.asarray(inputs["ln2_b"], dtype=np.float32)

    ISD = 1.0 / np.sqrt(D)
    wqf = (g1[:, None] * wq * ISD).astype(bfd)
    bqf = (lb1 @ wq + bq) * ISD
    wkf = (g1[:, None] * wkv[:, :D]).astype(bfd)
    bkf = lb1 @ wkv[:, :D] + bkv[:D]
    wvf = (g1[:, None] * wkv[:, D:]).astype(bfd)
    bvf = lb1 @ wkv[:, D:] + bkv[D:]
    w1f = (g2[:, None] * w1).astype(bfd)
    b1f = lb2 @ w1 + b1
    w2f = w2.astype(bfd)

    def dm(v):  # [D] -> [P, 8] dim-major chunk layout
        return np.ascontiguousarray(v.reshape(8, P).T)

    shared = {
        "wq": np.ascontiguousarray(wqf),
        "wkv": np.ascontiguousarray(
            np.concatenate([wkf, wvf], axis=1)),
        "w1": np.ascontiguousarray(w1f),
        "w2": np.ascontiguousarray(w2f),
        "bqs": dm(bqf),
        "bkvk": dm(bkf),
        "bvb": np.ascontiguousarray(
            np.broadcast_to(bvf, (P, D)).astype(np.float32)),
        "b1s": np.ascontiguousarray(b1f.reshape(32, P).T),
        "b2s": dm(b2),
        "qrow": np.ones((1, N_HEAD * WINDOW), dtype=bfd),
    }

    in_maps = []
    for core in range(8):
        b, c = core // 2, core % 2
        qidx = np.concatenate(
            [np.arange((2 * L + c) * P, (2 * L + c + 1) * P)
             for L in range(8)])
        xq = x[b, qidx]
        xw = x[b, S - WINDOW:S]
        pad = kpm[b, S - WINDOW:S]
        kaux = np.where(pad, np.float32(MASKVAL), np.float32(0.0))
        col = np.arange(256)[None, :]
        row = np.arange(P)[:, None]
        mq = (col <= c * P + row).astype(np.float32)
        m = dict(shared)
        m["xin"] = np.ascontiguousarray(
            np.concatenate([xq, xw], axis=0).astype(bfd))
        m["xinT"] = np.ascontiguousarray(xq.T.astype(bfd))
        m["krow"] = np.ascontiguousarray(
            np.tile(kaux, N_HEAD)[None, :].astype(bfd))
        m["maskq"] = np.ascontiguousarray(mq.astype(bfd))
        in_maps.append(m)
    return in_maps


def kernel(**inputs):
    from concourse.bass_utils import run_bass_kernel_spmd

    if "nc" not in _CACHE:
        _CACHE["nc"] = _build_program()
    nc = _CACHE["nc"]

    in_maps = _prep_inputs(inputs)
    trace = os.environ.get("KERNEL_TRACE", "0") == "1"
    res = run_bass_kernel_spmd(nc, in_maps, core_ids=list(range(8)),
                               trace=trace)
    if res.exec_time_ns is not None:
        print(f"HW exec time: {res.exec_time_ns} ns")
        _CACHE["exec_time_ns"] = res.exec_time_ns
    out = np.empty((B, S, D), dtype=np.float32)
    for core in range(8):
        b, c = core // 2, core % 2
        y = res.results[core]["y"].T  # [1024 local queries, D]
        yr = y.reshape(8, P, D)
        for L in range(8):
            out[b, (2 * L + c) * P:(2 * L + c + 1) * P] = yr[L]
    return out


# revision 26
# speedup vs baseline: 1.0006x; 1.0006x over previous
"""Trainium2 Bass kernel for a custom transformer block (v2).

Sharding: 8 cores = 4 batches x 2 query-interleave classes. Core (b, c)
owns global 128-query blocks gb = 2L + c (L = 0..7), so the causal
triangle splits evenly: per-L key-chunk counts CNT = [2,4,6,8,8,8,8,8]
are identical across cores and 18.75% of score/exp/AV work is skipped.

Key techniques vs v1:
- All matmuls in bf16 (weights pre-cast host-side; LN gains/biases and
  the 1/sqrt(D) scale folded into wq/wkv/w1 on the host).
- No PE transposes: XBAR DMA-transpose moves z -> zT, probs -> probsT,
  attention-out -> token-major, z2 -> z2T.
- Scores computed query-major [q, keys] per (head, L); key-pad mask
  folded into the score matmul via a 65th contraction row (q_aux = 1,
  k_aux = -80 * pad). Causal masking only on the two diagonal 128-key
  chunks per L < 4, as a {0,1} bf16 multiply after exp.
- No softmax division on the full tensor: per-(q, head) denominators
  are reduced from the bf16 probs (DVE) and applied as a single [P,1]
  scale on the 64-wide attention output (Pool).
- Engine split: PE matmuls; ACT exp/silu/QK evacs; DVE stats, dens,
  oa+h2 psum evacs; Pool SBUF-side applies/masks/normalize (GPSIMD
  cannot touch PSUM); SP issues all DMAs.
"""
import sys
import os

if "/opt/trn_rl_repo" not in sys.path:
    sys.path.insert(0, "/opt/trn_rl_repo")

import numpy as np
import ml_dtypes

B, S, D = 4, 2048, 1024
N_HEAD = 16
D_HEAD = 64
WINDOW = 1024
D_FF = 4096
EPS = 1e-5
P = 128
CNT = [2, 4, 6, 8, 8, 8, 8, 8]
MASKVAL = -80.0

_CACHE = {}


def _build_program():
    import concourse.bacc as bacc
    import concourse.mybir as mybir
    from concourse.tile import TileContext

    F32 = mybir.dt.float32
    BF16 = mybir.dt.bfloat16
    AF = mybir.ActivationFunctionType
    ALU = mybir.AluOpType
    AX = mybir.AxisListType

    nc = bacc.Bacc("TRN2", target_bir_lowering=False, debug=False,
                   num_devices=8)

    xin_d = nc.dram_tensor("xin", [2 * WINDOW, D], BF16, kind="ExternalInput")
    xinT_d = nc.dram_tensor("xinT", [D, WINDOW], BF16, kind="ExternalInput")
    wq_d = nc.dram_tensor("wq", [D, D], BF16, kind="ExternalInput")
    wkv_d = nc.dram_tensor("wkv", [D, 2 * D], BF16, kind="ExternalInput")
    w1_d = nc.dram_tensor("w1", [D, D_FF], BF16, kind="ExternalInput")
    w2_d = nc.dram_tensor("w2", [D_FF, D], BF16, kind="ExternalInput")
    bqs_d = nc.dram_tensor("bqs", [P, 8], F32, kind="ExternalInput")
    bkvk_d = nc.dram_tensor("bkvk", [P, 8], F32, kind="ExternalInput")
    bvb_d = nc.dram_tensor("bvb", [P, D], F32, kind="ExternalInput")
    b1s_d = nc.dram_tensor("b1s", [P, 32], F32, kind="ExternalInput")
    b2s_d = nc.dram_tensor("b2s", [P, 8], F32, kind="ExternalInput")
    qrow_d = nc.dram_tensor("qrow", [1, N_HEAD * WINDOW], BF16,
                            kind="ExternalInput")
    krow_d = nc.dram_tensor("krow", [1, N_HEAD * WINDOW], BF16,
                            kind="ExternalInput")
    maskq_d = nc.dram_tensor("maskq", [P, 256], BF16, kind="ExternalInput")
    y_d = nc.dram_tensor("y", [D, WINDOW], F32, kind="ExternalOutput")

    with TileContext(nc) as tc:
        cpool = tc.alloc_tile_pool(name="const", bufs=1, side="left")
        smallc = cpool.tile([P, 56], F32)
        bqs = smallc[:, 0:8]
        bkvk = smallc[:, 8:16]
        b1s = smallc[:, 16:48]
        b2s = smallc[:, 48:56]
        bvb = cpool.tile([P, D], F32)
        maskq = cpool.tile([P, 256], BF16)
        nc.sync.dma_start(bqs, bqs_d[:])
        nc.sync.dma_start(bkvk, bkvk_d[:])
        nc.sync.dma_start(b1s, b1s_d[:])
        nc.sync.dma_start(b2s, b2s_d[:])
        nc.sync.dma_start(bvb[:], bvb_d[:])
        nc.sync.dma_start(maskq[:], maskq_d[:])

        # ---------------- LN1 + DMA-transpose to dim-major -----------------
        zTp = tc.alloc_tile_pool(name="zT", bufs=1, side="left")
        zqT = zTp.tile([P, 8, WINDOW], BF16)
        zwT = zTp.tile([P, 8, WINDOW], BF16)
        xz = tc.alloc_tile_pool(name="xz", bufs=3, side="left")

        def ln1_tile(t):
            xt = xz.tile([P, D], BF16, tag="x")
            nc.sync.dma_start(xt[:], xin_d[t * P:(t + 1) * P, :])
            st = xz.tile([P, 8], F32, tag="stats")
            musum, mu, vsum = st[:, 0:1], st[:, 1:2], st[:, 2:3]
            veps, sdv, rstd = st[:, 4:5], st[:, 5:6], st[:, 6:7]
            nc.vector.reduce_sum(musum, xt[:], axis=AX.X)
            nc.vector.tensor_scalar_mul(mu, musum, 1.0 / D)
            ztmp = xz.tile([P, D], BF16, tag="ztmp")
            nc.vector.scalar_tensor_tensor(
                ztmp[:], xt[:], mu, xt[:],
                op0=ALU.subtract, op1=ALU.mult, accum_out=vsum)
            nc.vector.tensor_scalar(veps, vsum, 1.0 / D, EPS,
                                    op0=ALU.mult, op1=ALU.add)
            nc.scalar.sqrt(sdv, veps)
            nc.vector.reciprocal(rstd, sdv)
            z = xz.tile([P, D], BF16, tag="z")
            nc.vector.tensor_scalar(z[:], xt[:], mu, rstd,
                                    op0=ALU.subtract, op1=ALU.mult)
            if t < 8:
                dst = zqT[:, :, t * P:(t + 1) * P]
            else:
                dst = zwT[:, :, (t - 8) * P:(t - 7) * P]
            nc.sync.dma_start(dst, z[:], transpose=True)

        qkvp = tc.alloc_tile_pool(name="qkv", bufs=1, side="right")
        qT = qkvp.tile([P, N_HEAD, WINDOW], BF16)   # rows 0-63 dims, 64 ones
        kT = qkvp.tile([P, N_HEAD, WINDOW], BF16)   # rows 0-63 dims, 64 -80*pad
        V = qkvp.tile([P, 8, D], BF16)              # token-major

        wst = tc.alloc_tile_pool(name="wst", bufs=2, side="left")
        psC = tc.alloc_tile_pool(name="psC", bufs=4, space="PSUM")

        for t in range(8):
            ln1_tile(t)

        # Q: weights stationary -> psum [dims 128, tokens 512]
        # qh-outer so qh0 matmuls start after only 4 LN tiles
        wqrs = []
        for wh in range(2):
            wqr = wst.tile([P, 8, 512], BF16, tag="wchunk")
            for kc in range(8):
                nc.sync.dma_start(
                    wqr[:, kc, :],
                    wq_d[kc * P:(kc + 1) * P, wh * 512:(wh + 1) * 512])
            wqrs.append(wqr)
        for qh in range(2):
            for co in range(8):
                pp = psC.tile([P, 512], F32, tag="proj")
                for kc in range(8):
                    nc.tensor.matmul(
                        pp[:], wqrs[co // 4][:, kc, (co % 4) * P:(co % 4 + 1) * P],
                        zqT[:, kc, qh * 512:(qh + 1) * 512],
                        start=(kc == 0), stop=(kc == 7))
                for hh in range(2):
                    nc.scalar.activation(
                        qT[0:64, 2 * co + hh, qh * 512:(qh + 1) * 512],
                        pp[hh * 64:hh * 64 + 64, :], AF.Identity,
                        bias=bqs[hh * 64:hh * 64 + 64, co:co + 1],
                        scale=1.0)
        nc.sync.dma_start(qT[64:65, :, :], qrow_d[:])

        for t in range(8, 16):
            ln1_tile(t)

        # V: activations stationary -> psum [keys 128, dims 512]
        for vh in range(2):
            wvr = wst.tile([P, 8, 512], BF16, tag="wchunk")
            for kc in range(8):
                nc.sync.dma_start(
                    wvr[:, kc, :],
                    wkv_d[kc * P:(kc + 1) * P,
                          D + vh * 512:D + (vh + 1) * 512])
            for tt in range(8):
                pp = psC.tile([P, 512], F32, tag="proj")
                for kc in range(8):
                    nc.tensor.matmul(
                        pp[:], zwT[:, kc, tt * P:(tt + 1) * P],
                        wvr[:, kc, :],
                        start=(kc == 0), stop=(kc == 7))
                nc.vector.tensor_tensor(
                    V[:, tt, vh * 512:(vh + 1) * 512], pp[:],
                    bvb[:, vh * 512:(vh + 1) * 512], op=ALU.add)

        # K: weights stationary
        for wh in range(2):
            wkr = wst.tile([P, 8, 512], BF16, tag="wchunk")
            for kc in range(8):
                nc.sync.dma_start(
                    wkr[:, kc, :],
                    wkv_d[kc * P:(kc + 1) * P, wh * 512:(wh + 1) * 512])
            for co in range(wh * 4, wh * 4 + 4):
                for qh in range(2):
                    pp = psC.tile([P, 512], F32, tag="proj")
                    for kc in range(8):
                        nc.tensor.matmul(
                            pp[:], wkr[:, kc, (co % 4) * P:(co % 4 + 1) * P],
                            zwT[:, kc, qh * 512:(qh + 1) * 512],
                            start=(kc == 0), stop=(kc == 7))
                    for hh in range(2):
                        nc.scalar.activation(
                            kT[0:64, 2 * co + hh, qh * 512:(qh + 1) * 512],
                            pp[hh * 64:hh * 64 + 64, :], AF.Identity,
                            bias=bkvk[hh * 64:hh * 64 + 64, co:co + 1],
                            scale=1.0)
        nc.sync.dma_start(kT[64:65, :, :], krow_d[:])

        psC.release()
        wst.release()
        xz.release()
        zTp.release()

        # ---------------- attention -----------------------------------------
        attnp = tc.alloc_tile_pool(name="attn", bufs=1, side="left")
        attn = attnp.tile([P, 8, D], BF16)
        den = attnp.tile([P, 8, N_HEAD], F32)
        rinva = attnp.tile([P, 8, N_HEAD], F32)

        # MLP sc0 weight prefetch (PE can start MLP right after LN2)
        wf1 = tc.alloc_tile_pool(name="wf1", bufs=1, side="left")
        w1r0 = wf1.tile([P, 8, 1024], BF16, tag="w1r")
        for kc in range(8):
            nc.sync.dma_start(w1r0[:, kc, :],
                              w1_d[kc * P:(kc + 1) * P, 0:1024])

        pqp = tc.alloc_tile_pool(name="pq", bufs=4, side="left")
        ptp = tc.alloc_tile_pool(name="ptsT", bufs=4, side="left")
        oap = tc.alloc_tile_pool(name="oa", bufs=2, side="left")
        psS = tc.alloc_tile_pool(name="psS", bufs=3, space="PSUM")
        psA = tc.alloc_tile_pool(name="psA", bufs=2, space="PSUM")

        steps = [(hp, L) for hp in range(8) for L in range(8)]
        pts_saved = {}
        oa_tiles = {}

        def emit_scores(hp, L):
            nk = CNT[L] * P
            pq = pqp.tile([P, 2048], BF16, tag="pq")
            for hh in range(2):
                h = 2 * hp + hh
                sps = psS.tile([P, 1024], F32, tag="s")
                for half in range((nk + 511) // 512):
                    w = min(512, nk - half * 512)
                    nc.tensor.matmul(
                        sps[:, half * 512:half * 512 + w],
                        qT[0:65, h, L * P:(L + 1) * P],
                        kT[0:65, h, half * 512:half * 512 + w],
                        start=True, stop=True)
                if L >= 4:
                    # no causal mask: denominator rides the ACT accumulator
                    nc.scalar.activation(pq[:, hh * nk:hh * nk + nk],
                                         sps[:, 0:nk], AF.Exp,
                                         accum_out=den[:, L, h:h + 1])
                else:
                    nc.scalar.activation(pq[:, hh * nk:hh * nk + nk],
                                         sps[:, 0:nk], AF.Exp)
                    nc.vector.tensor_tensor(
                        pq[:, hh * nk + 2 * L * P:hh * nk + nk],
                        pq[:, hh * nk + 2 * L * P:hh * nk + nk],
                        maskq[:], op=ALU.mult)
                    nc.vector.reduce_sum(den[:, L, h:h + 1],
                                         pq[:, hh * nk:hh * nk + nk],
                                         axis=AX.X)
            pt = ptp.tile([P, 16, P], BF16, tag="pt")
            nc.sync.dma_start(pt[:, 0:2 * CNT[L], :], pq[:, 0:2 * nk],
                              transpose=True)
            pts_saved[(hp, L)] = pt

        def emit_av(hp, L):
            cnt = CNT[L]
            if L == 0:
                oa = oap.tile([P, WINDOW], BF16, tag="oa")
                oa_tiles[hp] = oa
            oa = oa_tiles[hp]
            pt = pts_saved.pop((hp, L))
            avp = psA.tile([P, P], F32, tag="av")
            for hh in range(2):
                h = 2 * hp + hh
                for kc in range(cnt):
                    nc.tensor.matmul(
                        avp[hh * 64:hh * 64 + 64, :],
                        V[:, kc, h * 64:(h + 1) * 64],
                        pt[:, cnt * hh + kc, :],
                        start=(kc == 0), stop=(kc == cnt - 1))
            nc.vector.tensor_copy(oa[:, L * P:(L + 1) * P], avp[:])
            if L == 7:
                nc.sync.dma_start(attn[:, :, hp * P:(hp + 1) * P], oa[:],
                                  transpose=True)
                # fold softmax 1/den into the attention tail per pair so the
                # LN2 window has no serial normalize block left
                nc.vector.reciprocal(rinva[:, :, 2 * hp:2 * hp + 2],
                                     den[:, :, 2 * hp:2 * hp + 2])
                for t in range(8):
                    for hh in range(2):
                        h = 2 * hp + hh
                        nc.vector.tensor_scalar_mul(
                            attn[:, t, h * 64:(h + 1) * 64],
                            attn[:, t, h * 64:(h + 1) * 64],
                            rinva[:, t, h:h + 1])

        LAG = 2
        for i, (hp, L) in enumerate(steps):
            emit_scores(hp, L)
            if i >= LAG:
                emit_av(*steps[i - LAG])
        for i in range(len(steps) - LAG, len(steps)):
            emit_av(*steps[i])

        psA.release()
        psS.release()
        oap.release()
        ptp.release()
        pqp.release()
        qkvp.release()

        # ---------------- normalize + LN2 + DMA-transpose -------------------
        z2Tp = tc.alloc_tile_pool(name="z2T", bufs=1, side="right")
        z2T = z2Tp.tile([P, 8, WINDOW], BF16)
        xz2 = tc.alloc_tile_pool(name="xz2", bufs=3, side="left")

        for t in range(8):
            at = attn[:, t, :]
            st = xz2.tile([P, 8], F32, tag="stats2")
            musum, mu, vsum = st[:, 0:1], st[:, 1:2], st[:, 2:3]
            veps, sdv, rstd = st[:, 4:5], st[:, 5:6], st[:, 6:7]
            nc.vector.reduce_sum(musum, at, axis=AX.X)
            nc.vector.tensor_scalar_mul(mu, musum, 1.0 / D)
            ztmp = xz2.tile([P, D], BF16, tag="z2tmp")
            nc.vector.scalar_tensor_tensor(
                ztmp[:], at, mu, at,
                op0=ALU.subtract, op1=ALU.mult, accum_out=vsum)
            nc.vector.tensor_scalar(veps, vsum, 1.0 / D, EPS,
                                    op0=ALU.mult, op1=ALU.add)
            nc.scalar.sqrt(sdv, veps)
            nc.vector.reciprocal(rstd, sdv)
            z2 = xz2.tile([P, D], BF16, tag="z2")
            nc.vector.tensor_scalar(z2[:], at, mu, rstd,
                                    op0=ALU.subtract, op1=ALU.mult)
            nc.sync.dma_start(z2T[:, :, t * P:(t + 1) * P], z2[:],
                              transpose=True)

        # ---------------- MLP ------------------------------------------------
        h2p = tc.alloc_tile_pool(name="h2acc", bufs=1, side="left")
        h2acc = h2p.tile([P, 8, WINDOW], F32)
        xinTp = tc.alloc_tile_pool(name="xinT", bufs=1, side="left")
        xinT = xinTp.tile([P, 8, WINDOW], BF16)
        nc.sync.dma_start(xinT[:], xinT_d.rearrange("(c p) n -> p c n", p=P))
        for co in range(8):
            nc.scalar.activation(h2acc[:, co, :], xinT[:, co, :],
                                 AF.Identity, bias=b2s[:, co:co + 1],
                                 scale=1.0)

        wf2 = tc.alloc_tile_pool(name="wf2", bufs=2, side="right")
        h1p = tc.alloc_tile_pool(name="h1p", bufs=1, side="left")
        psF1 = tc.alloc_tile_pool(name="psF1", bufs=4, space="PSUM")
        psF2 = tc.alloc_tile_pool(name="psF2", bufs=4, space="PSUM")

        for sc in range(4):
            if sc == 0:
                w1r = w1r0
            else:
                w1r = wf1.tile([P, 8, 1024], BF16, tag="w1r")
                for kc in range(8):
                    nc.sync.dma_start(
                        w1r[:, kc, :],
                        w1_d[kc * P:(kc + 1) * P, sc * 1024:(sc + 1) * 1024])
            h1 = h1p.tile([P, 8, WINDOW], BF16, tag="h1")
            for qh in range(2):
                for ft in range(8):
                    hp1 = psF1.tile([P, 512], F32, tag="h1ps")
                    for kc in range(8):
                        nc.tensor.matmul(
                            hp1[:], w1r[:, kc, ft * P:(ft + 1) * P],
                            z2T[:, kc, qh * 512:(qh + 1) * 512],
                            start=(kc == 0), stop=(kc == 7))
                    nc.scalar.activation(
                        h1[:, ft, qh * 512:(qh + 1) * 512], hp1[:], AF.Silu,
                        bias=b1s[:, sc * 8 + ft:sc * 8 + ft + 1], scale=1.0)
            w2r = wf2.tile([P, 8, 1024], BF16, tag="w2r")
            for kc in range(8):
                nc.sync.dma_start(
                    w2r[:, kc, :],
                    w2_d[(sc * 8 + kc) * P:(sc * 8 + kc + 1) * P, :])
            for co in range(8):
                for qh in range(2):
                    hp2 = psF2.tile([P, 512], F32, tag="h2ps")
                    for kc in range(8):
                        nc.tensor.matmul(
                            hp2[:], w2r[:, kc, co * P:(co + 1) * P],
                            h1[:, kc, qh * 512:(qh + 1) * 512],
                            start=(kc == 0), stop=(kc == 7))
                    dstp = h2acc[:, co, qh * 512:(qh + 1) * 512]
                    nc.vector.tensor_tensor(dstp, hp2[:], dstp, op=ALU.add)
                if sc == 3:
                    nc.sync.dma_start(y_d[co * P:(co + 1) * P, :],
                                      h2acc[:, co, :])

        psF2.release()
        psF1.release()
        h1p.release()
        wf2.release()
        xinTp.release()
        h2p.release()
        xz2.release()
        z2Tp.release()
        wf1.release()
        attnp.release()
        cpool.release()

    nc.compile()
    return nc


def _prep_inputs(inputs):
    bfd = ml_dtypes.bfloat16
    x = np.asarray(inputs["x"], dtype=np.float32)
    kpm = np.asarray(inputs["key_pad_mask"]).astype(bool)
    wq = np.asarray(inputs["wq"], dtype=np.float32)
    wkv = np.asarray(inputs["wkv"], dtype=np.float32)
    w1 = np.asarray(inputs["w1"], dtype=np.float32)
    w2 = np.asarray(inputs["w2"], dtype=np.float32)
    bq = np.asarray(inputs["bq"], dtype=np.float32)
    bkv = np.asarray(inputs["bkv"], dtype=np.float32)
    b1 = np.asarray(inputs["b1"], dtype=np.float32)
    b2 = np.asarray(inputs["b2"], dtype=np.float32)
    g1 = np.asarray(inputs["ln1_g"], dtype=np.float32)
    lb1 = np.asarray(inputs["ln1_b"], dtype=np.float32)
    g2 = np.asarray(inputs["ln2_g"], dtype=np.float32)
    lb2 = np.asarray(inputs["ln2_b"], dtype=np.float32)

    ISD = 1.0 / np.sqrt(D)
    wqf = (g1[:, None] * wq * ISD).astype(bfd)
    bqf = (lb1 @ wq + bq) * ISD
    wkf = (g1[:, None] * wkv[:, :D]).astype(bfd)
    bkf = lb1 @ wkv[:, :D] + bkv[:D]
    wvf = (g1[:, None] * wkv[:, D:]).astype(bfd)
    bvf = lb1 @ wkv[:, D:] + bkv[D:]
    w1f = (g2[:, None] * w1).astype(bfd)
    b1f = lb2 @ w1 + b1
    w2f = w2.astype(bfd)

    def dm(v):  # [D] -> [P, 8] dim-major chunk layout
        return np.ascontiguousarray(v.reshape(8, P).T)

    shared = {
        "wq": np.ascontiguousarray(wqf),
        "wkv": np.ascontiguousarray(
            np.concatenate([wkf, wvf], axis=1)),
        "w1": np.ascontiguousarray(w1f),
        "w2": np.ascontiguousarray(w2f),
        "bqs": dm(bqf),
        "bkvk": dm(bkf),
        "bvb": np.ascontiguousarray(
            np.broadcast_to(bvf, (P, D)).astype(np.float32)),
        "b1s": np.ascontiguousarray(b1f.reshape(32, P).T),
        "b2s": dm(b2),
        "qrow": np.ones((1, N_HEAD * WINDOW), dtype=bfd),
    }

    in_maps = []
    for core in range(8):
        b, c = core // 2, core % 2
        qidx = np.concatenate(
            [np.arange((2 * L + c) * P, (2 * L + c + 1) * P)
             for L in range(8)])
        xq = x[b, qidx]
        xw = x[b, S - WINDOW:S]
        pad = kpm[b, S - WINDOW:S]
        kaux = np.where(pad, np.float32(MASKVAL), np.float32(0.0))
        col = np.arange(256)[None, :]
        row = np.arange(P)[:, None]
        mq = (col <= c * P + row).astype(np.float32)
        m = dict(shared)
        m["xin"] = np.ascontiguousarray(
            np.concatenate([xq, xw], axis=0).astype(bfd))
        m["xinT"] = np.ascontiguousarray(xq.T.astype(bfd))
        m["krow"] = np.ascontiguousarray(
            np.tile(kaux, N_HEAD)[None, :].astype(bfd))
        m["maskq"] = np.ascontiguousarray(mq.astype(bfd))
        in_maps.append(m)
    return in_maps


def kernel(**inputs):
    from concourse.bass_utils import run_bass_kernel_spmd

    if "nc" not in _CACHE:
        _CACHE["nc"] = _build_program()
    nc = _CACHE["nc"]

    in_maps = _prep_inputs(inputs)
    trace = os.environ.get("KERNEL_TRACE", "0") == "1"
    res = run_bass_kernel_spmd(nc, in_maps, core_ids=list(range(8)),
                               trace=trace)
    if res.exec_time_ns is not None:
        print(f"HW exec time: {res.exec_time_ns} ns")
        _CACHE["exec_time_ns"] = res.exec_time_ns
    out = np.empty((B, S, D), dtype=np.float32)
    for core in range(8):
        b, c = core // 2, core % 2
        y = res.results[core]["y"].T  # [1024 local queries, D]
        yr = y.reshape(8, P, D)
        for L in range(8):
            out[b, (2 * L + c) * P:(2 * L + c + 1) * P] = yr[L]
    return out
